# revision 1
# baseline (speedup 1.0000x reference)
# Bass/Trainium2 kernel for nn_L2PairwiceObjectiveFunction (pairwise L2 loss
# between per-row linear interpolations of two curve sets onto a common
# uniform grid).
#
# Full inputs: x, y1, y2 [1024, 8192] f32 (x sorted per row).
# Output: [1024, 1024] f32.
#
# Sharding: batch rows split across 8 NeuronCores (128 rows each, rows on
# SBUF partitions). The pairwise bilinear form uses an AllGather of the
# transposed [3072, 128] interpolated y2 grids (bf16) followed by a local
# PE matmul.
#
# Interpolation algorithm (searchsorted-free): the common grid is UNIFORM,
# so each data point's grid cell is computable elementwise:
# c[n] = floor((x[n]-xmin)/dx) + 1, clipped to [0, 3000]. For grid point m
# the bracketing segment is the last n with c[n] <= m. We scatter per-datum
# quantities (cell marker, frac(x), gap, y-lo, y-next; int16-quantized)
# into grid bins with gpsimd local_scatter (true per-partition indices;
# last-datum-per-bin enforced by a dedup mask so indices are unique), then
# fill empty bins with a carry-forward tensor_tensor_scan
# (state = empty*state + value). Interpolation is then pure elementwise
# work. Bin space is processed in two scatter halves x two scan/interp
# quarters to fit SBUF.

import numpy as np

B, N, M, NCORES = 1024, 8192, 3000, 8
R = B // NCORES  # 128 rows per core
P = 128
NBINS = 3004        # 2*HBINS bins (c clipped to [0, 3000])
HBINS = 1502        # bins per scatter half: [0,1502), [1502,3004)
QBINS = 751         # bins per scan/interp quarter
NIDX = 4608         # datum window per half (covers Binomial spread at ~11 sigma)
WOFF = (0, N - NIDX)   # window starts per half
WPAD = NIDX + 16    # padded quant-tile width (need NIDX+1 for shifted reads)
USCALE = 32766.0
YSCALE = 3000.0
DXSCALE = 1e7
DXCLIP = 3.2e-3
KT = 24             # matmul k-tiles; grid padded 3000 -> 3072
MT = KT * P
WB = 512            # stage-A column block


def build_nc(xmin, xmax, debug=False):
    import concourse.bacc as bacc
    import concourse.mybir as mybir
    from concourse.tile import TileContext
    from concourse import library_config
    from concourse.tile_rust import add_dep_helper

    F32, BF16, I16 = mybir.dt.float32, mybir.dt.bfloat16, mybir.dt.int16
    A = mybir.AluOpType
    AF = mybir.ActivationFunctionType

    dx = float((np.float32(xmax) - np.float32(xmin)) / np.float32(M - 1))
    inv_dx = float(np.float32(1.0) / np.float32(dx))

    nc = bacc.Bacc("TRN2", target_bir_lowering=False)
    x_in = nc.dram_tensor("x", [R, N], F32, kind="ExternalInput")
    y1_in = nc.dram_tensor("y1", [R, N], F32, kind="ExternalInput")
    y2_in = nc.dram_tensor("y2", [R, N], F32, kind="ExternalInput")
    xc_in = nc.dram_tensor("xc", [1, M], F32, kind="ExternalInput")
    id_in = nc.dram_tensor("ident", [P, P], BF16, kind="ExternalInput")
    o_out = nc.dram_tensor("out", [R, B], F32, kind="ExternalOutput")
    dbg = {}
    if debug:
        for nm, w, dt in [
            ("d_cp1", WPAD, I16), ("d_u16", WPAD, I16), ("d_dxq", WPAD, I16),
            ("d_y1q", WPAD, I16), ("d_idx", NIDX, I16),
            ("d_fcp1", NBINS, F32), ("d_fu", NBINS, F32), ("d_fdx", NBINS, F32),
            ("d_fy1", NBINS, F32), ("d_fy1n", NBINS, F32),
            ("d_y1c", M, BF16), ("d_y2c", M, BF16), ("d_sq1", 1, F32),
        ]:
            dbg[nm] = nc.dram_tensor(nm, [R, w], dt, kind="ExternalOutput")

    with TileContext(nc) as tc:
        with (
            tc.tile_pool(name="pers", bufs=1) as pers,
            tc.tile_pool(name="psum", bufs=2, space="PSUM") as pp,
            tc.tile_pool(name="mmpsum", bufs=1, space="PSUM") as mmpp,
            tc.tile_pool(name="dram", bufs=1, space="DRAM") as dp,
        ):
            lib_bi = nc.gpsimd.load_library(library_config.local_scatter)

            x0 = pers.tile([P, 1], F32, tag="x0")
            xlast = pers.tile([P, 1], F32, tag="xlast")
            nc.sync.dma_start(out=x0[:], in_=x_in[:, 0:1])
            nc.sync.dma_start(out=xlast[:], in_=x_in[:, N - 1:N])
            negone = pers.tile([P, 1], I16, tag="negone")
            nc.vector.memset(negone[:], -1)

            y1c = pers.tile([P, MT], BF16, tag="y1c")
            y2c = pers.tile([P, MT], BF16, tag="y2c")
            nc.vector.memset(y1c[:, M:], 0)
            nc.vector.memset(y2c[:, M:], 0)
            sqacc = {}
            for ynm in ("y1", "y2"):
                s = pers.tile([P, 1], F32, tag=f"sqacc_{ynm}")
                nc.vector.memset(s[:], 0)
                sqacc[ynm] = s
            carries = {}   # latest scan carry [P,1] per array
            inits = {}     # scan initials from first datum
            ANAMES = ("cp1", "u", "dx", "y1", "y1n", "y2", "y2n")

            with (
                tc.tile_pool(name="qp", bufs=1) as qp,
                tc.tile_pool(name="sp", bufs=2) as sp,
            ):
                for h in range(2):
                    woff = WOFF[h]
                    # ---- stage A: quantize datum window [woff, woff+NIDX] ----
                    cp1 = qp.tile([P, WPAD], I16, tag="cp1")
                    u16 = qp.tile([P, WPAD], I16, tag="u16")
                    dxq = qp.tile([P, WPAD], I16, tag="dxq")
                    y1q = qp.tile([P, WPAD], I16, tag="y1q")
                    y2q = qp.tile([P, WPAD], I16, tag="y2q")
                    for t in (cp1, u16, dxq, y1q, y2q):
                        nc.vector.memset(t[:, NIDX:], 0)
                    for bi in range(NIDX // WB):
                        lo = woff + bi * WB
                        wext = WB + 1 if lo + WB < N else WB
                        sl = slice(bi * WB, bi * WB + WB)
                        xb = sp.tile([P, WB + 1], F32, tag="xb")
                        nc.sync.dma_start(out=xb[:, :wext],
                                          in_=x_in[:, lo:lo + wext])
                        if wext == WB:
                            nc.vector.memset(xb[:, WB:], 0)
                        # t5 = (x - xmin)/dx + 0.5, clipped to [-0.5, 3000.0]
                        t5 = sp.tile([P, WB], F32, tag="t5")
                        nc.scalar.activation(t5[:], xb[:, :WB], AF.Copy,
                                             bias=float(0.5 - xmin * inv_dx),
                                             scale=inv_dx)
                        nc.vector.tensor_scalar(out=t5[:], in0=t5[:],
                                                scalar1=3000.0, scalar2=-0.5,
                                                op0=A.min, op1=A.max)
                        # c16 = round(t5) = floor(t)+1
                        c16b = sp.tile([P, WB], I16, tag="c16b")
                        nc.vector.tensor_copy(out=c16b[:], in_=t5[:])
                        nc.vector.tensor_scalar(out=cp1[:, sl], in0=c16b[:],
                                                scalar1=1, scalar2=None, op0=A.add)
                        # u16 = round((t5 + 0.5 - c16) * USCALE)
                        cf = sp.tile([P, WB], F32, tag="cf")
                        nc.scalar.copy(out=cf[:], in_=c16b[:])
                        nc.vector.scalar_tensor_tensor(out=t5[:], in0=t5[:],
                                                       scalar=0.5, in1=cf[:],
                                                       op0=A.add, op1=A.subtract)
                        nc.scalar.activation(u16[:, sl], t5[:], AF.Copy,
                                             scale=USCALE)
                        # gap -> dxq
                        xd = sp.tile([P, WB], F32, tag="xd")
                        nc.vector.tensor_tensor(out=xd[:], in0=xb[:, 1:WB + 1],
                                                in1=xb[:, :WB], op=A.subtract)
                        nc.vector.tensor_scalar(out=dxq[:, sl], in0=xd[:],
                                                scalar1=DXCLIP, scalar2=DXSCALE,
                                                op0=A.min, op1=A.mult)
                        # y quantization
                        yb = sp.tile([P, WB], F32, tag="yb")
                        nc.sync.dma_start(out=yb[:], in_=y1_in[:, lo:lo + WB])
                        nc.scalar.activation(y1q[:, sl], yb[:], AF.Copy,
                                             scale=YSCALE)
                        yb2 = sp.tile([P, WB], F32, tag="yb")
                        nc.sync.dma_start(out=yb2[:], in_=y2_in[:, lo:lo + WB])
                        nc.scalar.activation(y2q[:, sl], yb2[:], AF.Copy,
                                             scale=YSCALE)
                    if h == 0:
                        # col NIDX (shifted reads): quantize datum NIDX
                        xe = sp.tile([P, 4], F32, tag="xe")
                        nc.sync.dma_start(out=xe[:, 0:1], in_=x_in[:, NIDX:NIDX + 1])
                        nc.sync.dma_start(out=xe[:, 1:2], in_=y1_in[:, NIDX:NIDX + 1])
                        nc.sync.dma_start(out=xe[:, 2:3], in_=y2_in[:, NIDX:NIDX + 1])
                        t5e = sp.tile([P, 1], F32, tag="t5e")
                        nc.scalar.activation(t5e[:], xe[:, 0:1], AF.Copy,
                                             bias=float(0.5 - xmin * inv_dx),
                                             scale=inv_dx)
                        nc.vector.tensor_scalar(out=t5e[:], in0=t5e[:],
                                                scalar1=3000.0, scalar2=-0.5,
                                                op0=A.min, op1=A.max)
                        c16e = sp.tile([P, 1], I16, tag="c16e")
                        nc.vector.tensor_copy(out=c16e[:], in_=t5e[:])
                        nc.vector.tensor_scalar(out=cp1[:, NIDX:NIDX + 1],
                                                in0=c16e[:], scalar1=1,
                                                scalar2=None, op0=A.add)
                        nc.scalar.activation(y1q[:, NIDX:NIDX + 1], xe[:, 1:2],
                                             AF.Copy, scale=YSCALE)
                        nc.scalar.activation(y2q[:, NIDX:NIDX + 1], xe[:, 2:3],
                                             AF.Copy, scale=YSCALE)
                        # scan initials from datum 0
                        for nm, src in [("cp1", cp1[:, 0:1]), ("u", u16[:, 0:1]),
                                        ("y1", y1q[:, 0:1]), ("y1n", y1q[:, 1:2]),
                                        ("y2", y2q[:, 0:1]), ("y2n", y2q[:, 1:2])]:
                            it = pers.tile([P, 1], F32, tag=f"init_{nm}")
                            nc.vector.tensor_copy(out=it[:], in_=src)
                            inits[nm] = it
                        inits["dx"] = 0.0

                    # ---- dedup + bin-index mask --------------------------
                    neq = qp.tile([P, NIDX], I16, tag="neq")
                    nc.vector.tensor_tensor(out=neq[:], in0=cp1[:, 0:NIDX],
                                            in1=cp1[:, 1:NIDX + 1], op=A.not_equal)
                    if h == 1:
                        nc.vector.memset(neq[:, NIDX - 1:], 0)
                    idx = qp.tile([P, NIDX], I16, tag="idx")
                    nc.vector.memset(idx[:], 0)
                    nc.vector.copy_predicated(out=idx[:], mask=neq[:],
                                              data=cp1[:, 0:NIDX])
                    nc.vector.tensor_scalar(out=idx[:], in0=idx[:], scalar1=1,
                                            scalar2=None, op0=A.subtract)
                    sel = qp.tile([P, NIDX], I16, tag="neq")  # reuse slot
                    if h == 0:
                        nc.vector.tensor_scalar(out=sel[:], in0=idx[:],
                                                scalar1=HBINS - 1, scalar2=None,
                                                op0=A.is_gt)
                        nc.vector.copy_predicated(
                            out=idx[:], mask=sel[:],
                            data=negone[:].to_broadcast([P, NIDX]))
                    else:
                        nc.vector.tensor_scalar(out=sel[:], in0=idx[:],
                                                scalar1=HBINS - 1, scalar2=None,
                                                op0=A.is_le)
                        nc.vector.tensor_scalar(out=idx[:], in0=idx[:],
                                                scalar1=HBINS, scalar2=None,
                                                op0=A.subtract)
                        nc.vector.copy_predicated(
                            out=idx[:], mask=sel[:],
                            data=negone[:].to_broadcast([P, NIDX]))

                    if debug and h == 0:
                        for nm, t in [("d_cp1", cp1), ("d_u16", u16),
                                      ("d_dxq", dxq), ("d_y1q", y1q),
                                      ("d_idx", idx)]:
                            nc.sync.dma_start(out=dbg[nm][:], in_=t[:])

                    # ---- scatters (7 arrays into this half's bins) -------
                    # local_scatter mishandles data APs with a nonzero offset
                    # (drops some writes), so the "next-datum" arrays are
                    # scattered with a materialized shifted INDEX array
                    # instead: value y[j] goes to the bin of datum j-1.
                    idxp = qp.tile([P, NIDX], I16, tag="idxp")
                    nc.vector.memset(idxp[:, 0:1], -1)
                    nc.vector.tensor_copy(out=idxp[:, 1:NIDX],
                                          in_=idx[:, 0:NIDX - 1])
                    adata = {
                        "cp1": (cp1[:, 0:NIDX], idx), "u": (u16[:, 0:NIDX], idx),
                        "dx": (dxq[:, 0:NIDX], idx),
                        "y1": (y1q[:, 0:NIDX], idx),
                        "y1n": (y1q[:, 0:NIDX], idxp),
                        "y2": (y2q[:, 0:NIDX], idx),
                        "y2n": (y2q[:, 0:NIDX], idxp),
                    }
                    dsts = {}
                    for nm in ANAMES:
                        data_ap, idx_t = adata[nm]
                        dst = qp.tile([P, HBINS + 2], I16, tag=f"dst_{nm}")
                        sc_bi = nc.gpsimd.local_scatter(
                            dst[:, 0:HBINS], data_ap, idx_t[:],
                            channels=P, num_elems=HBINS, num_idxs=NIDX)
                        add_dep_helper(sc_bi.ins, lib_bi.ins, sync=True,
                                       reason="lib before scatter")
                        dsts[nm] = dst

                    # ---- per quarter: fill scans + interpolation ---------
                    for qh in range(2):
                        qb0 = h * HBINS + qh * QBINS
                        qs = slice(qh * QBINS, (qh + 1) * QBINS)
                        emt = qp.tile([P, QBINS], F32, tag="emt")
                        nc.vector.tensor_scalar(out=emt[:],
                                                in0=dsts["cp1"][:, qs],
                                                scalar1=0, scalar2=None,
                                                op0=A.is_equal)
                        filled = {}
                        for nm in ANAMES:
                            f = qp.tile([P, QBINS], F32, tag=f"fill_{nm}")
                            init = inits[nm] if (h == 0 and qh == 0) else carries[nm]
                            init_ap = init if isinstance(init, float) else init[:, 0:1]
                            nc.vector.tensor_tensor_scan(
                                f[:], emt[:], dsts[nm][:, qs], init_ap,
                                A.mult, A.add)
                            filled[nm] = f
                            cy = pers.tile([P, 1], F32, tag=f"carry_{nm}")
                            nc.vector.tensor_copy(out=cy[:],
                                                  in_=f[:, QBINS - 1:QBINS])
                            carries[nm] = cy

                        if debug:
                            for dnm, key in [("d_fcp1", "cp1"), ("d_fu", "u"),
                                             ("d_fdx", "dx"), ("d_fy1", "y1"),
                                             ("d_fy1n", "y1n")]:
                                nc.sync.dma_start(
                                    out=dbg[dnm][:, qb0:qb0 + QBINS],
                                    in_=filled[key][:])

                        # interpolation over grid m in [qb0, min(qb0+QBINS, M))
                        W = min(qb0 + QBINS, M) - qb0
                        if W <= 0:
                            continue
                        fsl = slice(0, W)
                        xcb = qp.tile([P, QBINS], F32, tag="xcb")
                        nc.sync.dma_start(
                            out=xcb[:, :W],
                            in_=xc_in[:, qb0:qb0 + W].to_broadcast([P, W]))
                        ma = qp.tile([P, QBINS], F32, tag="ma")
                        nc.vector.tensor_scalar(out=ma[:, :W], in0=xcb[:, :W],
                                                scalar1=x0[:, 0:1], scalar2=None,
                                                op0=A.is_ge)
                        scr1 = qp.tile([P, QBINS], F32, tag="scr1")
                        nc.vector.tensor_scalar(out=scr1[:, :W], in0=xcb[:, :W],
                                                scalar1=xlast[:, 0:1],
                                                scalar2=None, op0=A.is_le)
                        nc.vector.scalar_tensor_tensor(
                            out=ma[:, :W], in0=ma[:, :W],
                            scalar=float(1.0 / YSCALE), in1=scr1[:, :W],
                            op0=A.mult, op1=A.mult)
                        # x_lo = xmin + (cp1f - 2 + u)*dx ; us <- xc - x_lo
                        us = qp.tile([P, QBINS], F32, tag="us")
                        nc.scalar.activation(us[:, :W], filled["u"][:, fsl],
                                             AF.Copy, scale=float(dx / USCALE))
                        nc.vector.scalar_tensor_tensor(
                            out=us[:, :W], in0=filled["cp1"][:, fsl], scalar=dx,
                            in1=us[:, :W], op0=A.mult, op1=A.add)
                        nc.vector.scalar_tensor_tensor(
                            out=us[:, :W], in0=xcb[:, :W],
                            scalar=float(xmin - 2.0 * dx), in1=us[:, :W],
                            op0=A.subtract, op1=A.subtract)
                        # denom -> scr1b, recip -> scr2
                        scr1b = qp.tile([P, QBINS], F32, tag="scr1")
                        nc.vector.tensor_scalar(out=scr1b[:, :W],
                                                in0=filled["dx"][:, fsl],
                                                scalar1=0.0, scalar2=None,
                                                op0=A.is_equal)
                        nc.vector.scalar_tensor_tensor(
                            out=scr1b[:, :W], in0=filled["dx"][:, fsl],
                            scalar=float(1.0 / DXSCALE), in1=scr1b[:, :W],
                            op0=A.mult, op1=A.add)
                        nc.vector.tensor_scalar(out=scr1b[:, :W],
                                                in0=scr1b[:, :W],
                                                scalar1=1e-9, scalar2=None,
                                                op0=A.add)
                        scr2 = qp.tile([P, QBINS], F32, tag="scr2")
                        nc.vector.reciprocal(scr2[:, :W], scr1b[:, :W])
                        w_t = qp.tile([P, QBINS], F32, tag="w_t")
                        nc.vector.tensor_tensor(out=w_t[:, :W], in0=us[:, :W],
                                                in1=scr2[:, :W], op=A.mult)
                        nc.vector.tensor_scalar(out=w_t[:, :W], in0=w_t[:, :W],
                                                scalar1=1.0, scalar2=0.0,
                                                op0=A.min, op1=A.max)
                        for ynm, yc in [("y1", y1c), ("y2", y2c)]:
                            e = qp.tile([P, QBINS], F32, tag="scr2")
                            nc.vector.tensor_tensor(out=e[:, :W],
                                                    in0=filled[ynm + "n"][:, fsl],
                                                    in1=filled[ynm][:, fsl],
                                                    op=A.subtract)
                            nc.vector.tensor_tensor(out=e[:, :W], in0=w_t[:, :W],
                                                    in1=e[:, :W], op=A.mult)
                            nc.vector.tensor_tensor(out=e[:, :W], in0=e[:, :W],
                                                    in1=filled[ynm][:, fsl],
                                                    op=A.add)
                            nc.vector.tensor_tensor(out=yc[:, qb0:qb0 + W],
                                                    in0=e[:, :W], in1=ma[:, :W],
                                                    op=A.mult)
                            spt = sp.tile([P, 1], F32, tag="spt")
                            e2 = qp.tile([P, QBINS], F32, tag="scr2")
                            nc.scalar.activation(e2[:, :W], yc[:, qb0:qb0 + W],
                                                 AF.Square, accum_out=spt[:, 0:1])
                            nc.vector.tensor_tensor(out=sqacc[ynm][:],
                                                    in0=sqacc[ynm][:],
                                                    in1=spt[:], op=A.add)

            # ---- sq = mean(y^2) ------------------------------------------
            sqa = {}
            for ynm in ("y1", "y2"):
                s = pers.tile([P, 1], F32, tag=f"sqa_{ynm}")
                nc.vector.tensor_scalar(out=s[:], in0=sqacc[ynm][:],
                                        scalar1=float(1.0 / M), scalar2=None,
                                        op0=A.mult)
                sqa[ynm] = s

            if debug:
                nc.sync.dma_start(out=dbg["d_y1c"][:], in_=y1c[:, 0:M])
                nc.sync.dma_start(out=dbg["d_y2c"][:], in_=y2c[:, 0:M])
                nc.sync.dma_start(out=dbg["d_sq1"][:], in_=sqa["y1"][:])

            with (
                tc.tile_pool(name="ep", bufs=1) as ep,
                tc.tile_pool(name="rhsp", bufs=3) as rhsp,
            ):
                # ---- transposes to [m, rows] bf16 ------------------------
                ident = ep.tile([P, P], BF16, tag="ident")
                nc.sync.dma_start(out=ident[:], in_=id_in[:])
                y1T = ep.tile([P, MT], BF16, tag="y1T")
                y2T = ep.tile([P, MT], BF16, tag="y2T")
                for kt in range(KT):
                    for src, dstt in [(y1c, y1T), (y2c, y2T)]:
                        ps = pp.tile([P, P], BF16, tag="tps", space="PSUM")
                        nc.tensor.transpose(out=ps[:],
                                            in_=src[:, kt * P:(kt + 1) * P],
                                            identity=ident[:])
                        nc.vector.tensor_copy(out=dstt[:, kt * P:(kt + 1) * P],
                                              in_=ps[:])

                # ---- AllGather of y2T + sq2 hi/res (bf16) ----------------
                sq2hi = ep.tile([P, 1], BF16, tag="sq2hi")
                nc.vector.tensor_copy(out=sq2hi[:], in_=sqa["y2"][:])
                sq2hf = ep.tile([P, 1], F32, tag="sq2hf")
                nc.vector.tensor_copy(out=sq2hf[:], in_=sq2hi[:])
                sq2res = ep.tile([P, 1], BF16, tag="sq2res")
                nc.vector.tensor_tensor(out=sq2res[:], in0=sqa["y2"][:],
                                        in1=sq2hf[:], op=A.subtract)
                AGW = MT + 2  # 3074 per partition-row
                agin = dp.tile([P, AGW], BF16)
                agout = dp.tile([NCORES * P, AGW], BF16, addr_space="Shared")
                nc.sync.dma_start(out=agin[:, 0:MT], in_=y2T[:])
                nc.sync.dma_start(out=agin[:, MT:MT + 1], in_=sq2hi[:])
                nc.sync.dma_start(out=agin[:, MT + 1:AGW], in_=sq2res[:])
                nc.gpsimd.collective_compute(
                    "AllGather", A.bypass,
                    replica_groups=[list(range(NCORES))],
                    ins=[agin[:].opt()], outs=[agout[:].opt()])

                # ---- matmul: cross[i,j] = sum_m y1c[i,m]*y2c_all[j,m] ----
                cross = mmpp.tile([P, B], F32, space="PSUM")
                agv = agout[:].rearrange("(r p) f -> r p f", r=NCORES)
                for kt in range(KT):
                    rhs = rhsp.tile([P, B], BF16, tag="rhs")
                    nc.sync.dma_start(
                        out=rhs[:].rearrange("p (r f) -> p r f", r=NCORES),
                        in_=agv[:, :, kt * P:(kt + 1) * P]
                            .rearrange("r p f -> p r f"))
                    for jh in range(2):
                        nc.tensor.matmul(
                            cross[:, jh * 512:(jh + 1) * 512],
                            y1T[:, kt * P:(kt + 1) * P],
                            rhs[:, jh * 512:(jh + 1) * 512],
                            start=(kt == 0), stop=(kt == KT - 1))

                # ---- epilogue --------------------------------------------
                sq2hi_b = ep.tile([P, B], BF16, tag="sq2hi_b")
                nc.sync.dma_start(
                    out=sq2hi_b[:].rearrange("p (r f) -> p r f", r=NCORES),
                    in_=agv[:, :, MT:MT + 1].rearrange("r p f -> f r p")
                        .to_broadcast([P, NCORES, P]))
                sq2res_b = ep.tile([P, B], BF16, tag="sq2res_b")
                nc.sync.dma_start(
                    out=sq2res_b[:].rearrange("p (r f) -> p r f", r=NCORES),
                    in_=agv[:, :, MT + 1:MT + 2].rearrange("r p f -> f r p")
                        .to_broadcast([P, NCORES, P]))
                sq2g = ep.tile([P, B], F32, tag="sq2g")
                nc.vector.tensor_tensor(out=sq2g[:], in0=sq2hi_b[:],
                                        in1=sq2res_b[:], op=A.add)
                diff = ep.tile([P, B], F32, tag="diff")
                nc.vector.scalar_tensor_tensor(out=diff[:], in0=cross[:],
                                               scalar=float(-2.0 / M),
                                               in1=sq2g[:], op0=A.mult,
                                               op1=A.add)
                nc.vector.tensor_scalar(out=diff[:], in0=diff[:],
                                        scalar1=sqa["y1"][:, 0:1],
                                        scalar2=0.0, op0=A.add, op1=A.max)
                base = ep.tile([P, 1], F32, tag="base")
                nc.vector.tensor_tensor(out=base[:], in0=sqa["y1"][:],
                                        in1=sqa["y2"][:], op=A.add)
                nc.vector.tensor_scalar(out=base[:], in0=base[:], scalar1=1e-8,
                                        scalar2=None, op0=A.add)
                rbase = ep.tile([P, 1], F32, tag="rbase")
                nc.vector.reciprocal(rbase[:], base[:])
                nc.vector.scalar_tensor_tensor(out=diff[:], in0=diff[:],
                                               scalar=2.0,
                                               in1=rbase[:].to_broadcast([P, B]),
                                               op0=A.mult, op1=A.mult)
                lout = ep.tile([P, B], F32, tag="lout")
                nc.scalar.activation(lout[:], diff[:], AF.Sqrt)
                nc.sync.dma_start(out=o_out[:], in_=lout[:])

    nc.compile()
    return nc


def _host_prep(x):
    xmin = np.float32(x[:, 0].min())
    xmax = np.float32(x[:, -1].max())
    grid = np.linspace(np.float32(0.0), np.float32(1.0), M, dtype=np.float32)
    xc = (xmin + grid * (xmax - xmin)).astype(np.float32)[None, :]
    return xmin, xmax, xc


def kernel(x, y1, y2, debug=False, trace=False):
    import ml_dtypes
    from concourse.bass_utils import run_bass_kernel_spmd

    x = np.ascontiguousarray(x, dtype=np.float32)
    y1 = np.ascontiguousarray(y1, dtype=np.float32)
    y2 = np.ascontiguousarray(y2, dtype=np.float32)
    xmin, xmax, xc = _host_prep(x)
    ident = np.eye(P, dtype=ml_dtypes.bfloat16)

    nc = build_nc(float(xmin), float(xmax), debug=debug)
    in_maps = []
    for r in range(NCORES):
        rows = slice(r * R, (r + 1) * R)
        in_maps.append({"x": x[rows], "y1": y1[rows], "y2": y2[rows],
                        "xc": xc, "ident": ident})
    res = run_bass_kernel_spmd(nc, in_maps, core_ids=list(range(NCORES)),
                               trace=trace)
    out = np.concatenate([res.results[r]["out"] for r in range(NCORES)], axis=0)
    if debug or trace:
        return out, res
    return out



# revision 11
# speedup vs baseline: 1.8683x; 1.8683x over previous
# Bass/Trainium2 kernel for nn_L2PairwiceObjectiveFunction (pairwise L2 loss
# between per-row linear interpolations of two curve sets onto a common
# uniform grid).
#
# Full inputs: x, y1, y2 [1024, 8192] f32 (x sorted per row).
# Output: [1024, 1024] f32.
#
# Sharding: batch rows split across 8 NeuronCores (128 rows each, rows on
# SBUF partitions). The pairwise bilinear form uses an AllGather of the
# transposed [3072, 128] interpolated y2 grids (bf16) followed by a local
# PE matmul.
#
# Interpolation algorithm (searchsorted-free): the common grid is UNIFORM,
# so each data point's grid cell is computable elementwise:
# c[n] = floor((x[n]-xmin)/dx) + 1, clipped to [0, 3000]. For grid point m
# the bracketing segment is the last n with c[n] <= m. We scatter per-datum
# quantities (frac(x) offset, gap, y-lo, y-next; int16-quantized) into
# grid bins with gpsimd local_scatter (true per-partition indices;
# last-datum-per-bin enforced by a dedup mask so indices are unique), then
# fill empty bins with a carry-forward tensor_tensor_scan
# (state = empty*state + value). The offs scan adds dx per empty bin so it
# directly yields the interpolation numerator xc_m - x_lo. Interpolation
# is then pure elementwise work. Bin space is processed in two scatter
# halves x two scan/interp quarters to fit SBUF.

import numpy as np

B, N, M, NCORES = 1024, 8192, 3000, 8
R = B // NCORES  # 128 rows per core
P = 128
NBINS = 3004        # 2*HBINS bins (c clipped to [0, 3000])
HBINS = 1502        # bins per scatter half: [0,1502), [1502,3004)
QBINS = 751         # bins per scan/interp quarter
NIDX = 4608         # datum window per half (covers Binomial spread at ~11 sigma)
WOFF = (0, N - NIDX)   # window starts per half
WPAD = NIDX + 16    # padded quant-tile width (need NIDX+1 for shifted reads)
OSCALE = 30000.0    # frac-offset quant scale (payload = u*OSCALE + 1; 0 = empty)
YSCALE = 3000.0
DXSCALE = 1e7
DXCLIP = 3.2e-3
KT = 24             # matmul k-tiles; grid padded 3000 -> 3072
MT = KT * P
WB = 512            # stage-A column block


def build_nc(xmin, xmax, debug=False):
    import concourse.bacc as bacc
    import concourse.mybir as mybir
    from concourse.tile import TileContext
    from concourse import library_config
    from concourse.tile_rust import add_dep_helper

    F32, BF16, I16 = mybir.dt.float32, mybir.dt.bfloat16, mybir.dt.int16
    A = mybir.AluOpType
    AF = mybir.ActivationFunctionType

    dx = float((np.float32(xmax) - np.float32(xmin)) / np.float32(M - 1))
    inv_dx = float(np.float32(1.0) / np.float32(dx))

    nc = bacc.Bacc("TRN2", target_bir_lowering=False)
    x_in = nc.dram_tensor("x", [R, N], F32, kind="ExternalInput")
    y1_in = nc.dram_tensor("y1", [R, N], F32, kind="ExternalInput")
    y2_in = nc.dram_tensor("y2", [R, N], F32, kind="ExternalInput")
    xc_in = nc.dram_tensor("xc", [1, M], F32, kind="ExternalInput")
    id_in = nc.dram_tensor("ident", [P, P], BF16, kind="ExternalInput")
    o_out = nc.dram_tensor("out", [R, B], F32, kind="ExternalOutput")
    dbg = {}
    if debug:
        for nm, w, dt in [
            ("d_cc", WPAD, I16), ("d_offs", WPAD, I16), ("d_dxq", WPAD, I16),
            ("d_y1q", WPAD, I16), ("d_idx", NIDX, I16),
            ("d_fnum", NBINS, F32), ("d_fdx", NBINS, F32),
            ("d_fy1", NBINS, F32), ("d_fy1n", NBINS, F32),
            ("d_y1c", M, BF16), ("d_y2c", M, BF16), ("d_sq1", 1, F32),
        ]:
            dbg[nm] = nc.dram_tensor(nm, [R, w], dt, kind="ExternalOutput")

    with TileContext(nc) as tc:
        with (
            tc.tile_pool(name="pers", bufs=1) as pers,
            tc.tile_pool(name="psum", bufs=2, space="PSUM") as pp,
            tc.tile_pool(name="mmpsum", bufs=1, space="PSUM") as mmpp,
            tc.tile_pool(name="dram", bufs=1, space="DRAM") as dp,
        ):
            lib_bi = nc.gpsimd.load_library(library_config.local_scatter)

            x0 = pers.tile([P, 1], F32, tag="x0")
            xlast = pers.tile([P, 1], F32, tag="xlast")
            nc.sync.dma_start(out=x0[:], in_=x_in[:, 0:1])
            nc.sync.dma_start(out=xlast[:], in_=x_in[:, N - 1:N])
            negone = pers.tile([P, 1], I16, tag="negone")
            nc.vector.memset(negone[:], -1)

            y1c = pers.tile([P, MT], BF16, tag="y1c")
            y2c = pers.tile([P, MT], BF16, tag="y2c")
            nc.vector.memset(y1c[:, M:], 0)
            nc.vector.memset(y2c[:, M:], 0)
            sqacc = {}
            for ynm in ("y1", "y2"):
                s = pers.tile([P, 1], F32, tag=f"sqacc_{ynm}")
                nc.vector.memset(s[:], 0)
                sqacc[ynm] = s
            carries = {}   # latest scan carry [P,1] per array
            inits = {}     # scan initials from first datum
            ANAMES = ("offs", "dx", "y1", "y1n", "y2", "y2n")

            with (
                tc.tile_pool(name="qp", bufs=1) as qp,
                tc.tile_pool(name="sp", bufs=2) as sp,
            ):
                for h in range(2):
                    woff = WOFF[h]
                    # ---- stage A: quantize datum window [woff, woff+NIDX] ----
                    cc = qp.tile([P, WPAD], I16, tag="cc")
                    offs = qp.tile([P, WPAD], I16, tag="offs")
                    dxq = qp.tile([P, WPAD], I16, tag="dxq")
                    y1q = qp.tile([P, WPAD], I16, tag="y1q")
                    y2q = qp.tile([P, WPAD], I16, tag="y2q")
                    for t in (cc, offs, dxq, y1q, y2q):
                        nc.vector.memset(t[:, NIDX:], 0)
                    for bi in range(NIDX // WB):
                        lo = woff + bi * WB
                        wext = WB + 1 if lo + WB < N else WB
                        sl = slice(bi * WB, bi * WB + WB)
                        xb = sp.tile([P, WB + 1], F32, tag="xb")
                        nc.sync.dma_start(out=xb[:, :wext],
                                          in_=x_in[:, lo:lo + wext])
                        if wext == WB:
                            nc.vector.memset(xb[:, WB:], 0)
                        # t5 = (x - xmin)/dx + 0.5, clipped to [-0.5, 3000.0]
                        t5 = sp.tile([P, WB], F32, tag="t5")
                        nc.scalar.activation(t5[:], xb[:, :WB], AF.Copy,
                                             bias=float(0.5 - xmin * inv_dx),
                                             scale=inv_dx)
                        nc.vector.tensor_scalar(out=t5[:], in0=t5[:],
                                                scalar1=3000.0, scalar2=-0.5,
                                                op0=A.min, op1=A.max)
                        # cell c = round(t5) = floor(t)+1
                        nc.vector.tensor_copy(out=cc[:, sl], in_=t5[:])
                        # offs = round((t5 + 0.5 - c) * OSCALE) + 1  (0 = empty)
                        cf = sp.tile([P, WB], F32, tag="cf")
                        nc.scalar.copy(out=cf[:], in_=cc[:, sl])
                        nc.vector.scalar_tensor_tensor(out=t5[:], in0=t5[:],
                                                       scalar=0.5, in1=cf[:],
                                                       op0=A.add, op1=A.subtract)
                        nc.scalar.activation(offs[:, sl], t5[:], AF.Copy,
                                             scale=OSCALE, bias=1.0)
                        # gap -> dxq
                        xd = sp.tile([P, WB], F32, tag="xd")
                        nc.vector.tensor_tensor(out=xd[:], in0=xb[:, 1:WB + 1],
                                                in1=xb[:, :WB], op=A.subtract)
                        nc.vector.tensor_scalar(out=dxq[:, sl], in0=xd[:],
                                                scalar1=DXCLIP, scalar2=DXSCALE,
                                                op0=A.min, op1=A.mult)
                        # y quantization
                        yb = sp.tile([P, WB], F32, tag="yb")
                        nc.sync.dma_start(out=yb[:], in_=y1_in[:, lo:lo + WB])
                        nc.scalar.activation(y1q[:, sl], yb[:], AF.Copy,
                                             scale=YSCALE)
                        yb2 = sp.tile([P, WB], F32, tag="yb")
                        nc.sync.dma_start(out=yb2[:], in_=y2_in[:, lo:lo + WB])
                        nc.scalar.activation(y2q[:, sl], yb2[:], AF.Copy,
                                             scale=YSCALE)
                    if h == 0:
                        # col NIDX (shifted reads): quantize datum NIDX
                        xe = sp.tile([P, 4], F32, tag="xe")
                        nc.sync.dma_start(out=xe[:, 0:1], in_=x_in[:, NIDX:NIDX + 1])
                        t5e = sp.tile([P, 1], F32, tag="t5e")
                        nc.scalar.activation(t5e[:], xe[:, 0:1], AF.Copy,
                                             bias=float(0.5 - xmin * inv_dx),
                                             scale=inv_dx)
                        nc.vector.tensor_scalar(out=t5e[:], in0=t5e[:],
                                                scalar1=3000.0, scalar2=-0.5,
                                                op0=A.min, op1=A.max)
                        nc.vector.tensor_copy(out=cc[:, NIDX:NIDX + 1], in_=t5e[:])
                        # scan initials from datum 0
                        for nm, src in [("y1", y1q[:, 0:1]), ("y1n", y1q[:, 1:2]),
                                        ("y2", y2q[:, 0:1]), ("y2n", y2q[:, 1:2])]:
                            it = pers.tile([P, 1], F32, tag=f"init_{nm}")
                            nc.vector.tensor_copy(out=it[:], in_=src)
                            inits[nm] = it
                        inits["dx"] = 0.0
                        inits["offs"] = 0.0

                    # ---- dedup + bin-index mask --------------------------
                    neq = qp.tile([P, NIDX], I16, tag="neq")
                    nc.vector.tensor_tensor(out=neq[:], in0=cc[:, 0:NIDX],
                                            in1=cc[:, 1:NIDX + 1], op=A.not_equal)
                    if h == 1:
                        nc.vector.memset(neq[:, NIDX - 1:], 0)
                    idx = qp.tile([P, NIDX], I16, tag="idx")
                    nc.vector.memset(idx[:], -1)
                    nc.vector.copy_predicated(out=idx[:], mask=neq[:],
                                              data=cc[:, 0:NIDX])
                    sel = qp.tile([P, NIDX], I16, tag="neq")  # reuse slot
                    if h == 0:
                        nc.vector.tensor_scalar(out=sel[:], in0=idx[:],
                                                scalar1=HBINS - 1, scalar2=None,
                                                op0=A.is_gt)
                        nc.vector.copy_predicated(
                            out=idx[:], mask=sel[:],
                            data=negone[:].to_broadcast([P, NIDX]))
                    else:
                        nc.vector.tensor_scalar(out=sel[:], in0=idx[:],
                                                scalar1=HBINS - 1, scalar2=None,
                                                op0=A.is_le)
                        nc.vector.tensor_scalar(out=idx[:], in0=idx[:],
                                                scalar1=HBINS, scalar2=None,
                                                op0=A.subtract)
                        nc.vector.copy_predicated(
                            out=idx[:], mask=sel[:],
                            data=negone[:].to_broadcast([P, NIDX]))

                    if debug and h == 0:
                        for nm, t in [("d_cc", cc), ("d_offs", offs),
                                      ("d_dxq", dxq), ("d_y1q", y1q),
                                      ("d_idx", idx)]:
                            nc.sync.dma_start(out=dbg[nm][:], in_=t[:])

                    # ---- scatters (7 arrays into this half's bins) -------
                    # local_scatter mishandles data APs with a nonzero offset
                    # (drops some writes), so the "next-datum" arrays are
                    # scattered with a materialized shifted INDEX array
                    # instead: value y[j] goes to the bin of datum j-1.
                    idxp = qp.tile([P, NIDX], I16, tag="idxp")
                    nc.vector.memset(idxp[:, 0:1], -1)
                    nc.vector.tensor_copy(out=idxp[:, 1:NIDX],
                                          in_=idx[:, 0:NIDX - 1])
                    adata = {
                        "offs": (offs[:, 0:NIDX], idx),
                        "dx": (dxq[:, 0:NIDX], idx),
                        "y1": (y1q[:, 0:NIDX], idx),
                        "y1n": (y1q[:, 0:NIDX], idxp),
                        "y2": (y2q[:, 0:NIDX], idx),
                        "y2n": (y2q[:, 0:NIDX], idxp),
                    }
                    dsts = {}
                    for nm in ANAMES:
                        data_ap, idx_t = adata[nm]
                        dst = qp.tile([P, HBINS + 2], I16, tag=f"dst_{nm}")
                        sc_bi = nc.gpsimd.local_scatter(
                            dst[:, 0:HBINS], data_ap, idx_t[:],
                            channels=P, num_elems=HBINS, num_idxs=NIDX)
                        add_dep_helper(sc_bi.ins, lib_bi.ins, sync=True,
                                       reason="lib before scatter")
                        dsts[nm] = dst

                    # ---- per quarter: fill scans + interpolation ---------
                    for qh in range(2):
                        qb0 = h * HBINS + qh * QBINS
                        qs = slice(qh * QBINS, (qh + 1) * QBINS)
                        emt = qp.tile([P, QBINS], F32, tag="emt")
                        nc.vector.tensor_scalar(out=emt[:],
                                                in0=dsts["offs"][:, qs],
                                                scalar1=0, scalar2=None,
                                                op0=A.is_equal)
                        # num-scan input: filled bin -> dx - offs (grid pt to
                        # x_lo distance); empty bin -> dx (carry grows by dx)
                        nin = qp.tile([P, QBINS], F32, tag="nin")
                        nc.vector.tensor_scalar(out=nin[:],
                                                in0=dsts["offs"][:, qs],
                                                scalar1=float(-dx / OSCALE),
                                                scalar2=float(dx * (1.0 + 1.0 / OSCALE)),
                                                op0=A.mult, op1=A.add)
                        filled = {}
                        for nm in ANAMES:
                            f = qp.tile([P, QBINS], F32, tag=f"fill_{nm}")
                            init = inits[nm] if (h == 0 and qh == 0) else carries[nm]
                            init_ap = init if isinstance(init, float) else init[:, 0:1]
                            in1 = nin[:] if nm == "offs" else dsts[nm][:, qs]
                            nc.vector.tensor_tensor_scan(
                                f[:], emt[:], in1, init_ap,
                                A.mult, A.add)
                            filled[nm] = f
                            cy = pers.tile([P, 1], F32, tag=f"carry_{nm}")
                            nc.vector.tensor_copy(out=cy[:],
                                                  in_=f[:, QBINS - 1:QBINS])
                            carries[nm] = cy

                        if debug:
                            for dnm, key in [("d_fnum", "offs"),
                                             ("d_fdx", "dx"), ("d_fy1", "y1"),
                                             ("d_fy1n", "y1n")]:
                                nc.sync.dma_start(
                                    out=dbg[dnm][:, qb0:qb0 + QBINS],
                                    in_=filled[key][:])

                        # interpolation over grid m in [qb0, min(qb0+QBINS, M))
                        W = min(qb0 + QBINS, M) - qb0
                        if W <= 0:
                            continue
                        fsl = slice(0, W)
                        xcb = qp.tile([P, QBINS], F32, tag="xcb")
                        nc.sync.dma_start(
                            out=xcb[:, :W],
                            in_=xc_in[:, qb0:qb0 + W].to_broadcast([P, W]))
                        ma = qp.tile([P, QBINS], F32, tag="ma")
                        nc.vector.tensor_scalar(out=ma[:, :W], in0=xcb[:, :W],
                                                scalar1=x0[:, 0:1], scalar2=None,
                                                op0=A.is_ge)
                        scr1 = qp.tile([P, QBINS], F32, tag="scr1")
                        nc.vector.tensor_scalar(out=scr1[:, :W], in0=xcb[:, :W],
                                                scalar1=xlast[:, 0:1],
                                                scalar2=None, op0=A.is_le)
                        nc.vector.scalar_tensor_tensor(
                            out=ma[:, :W], in0=ma[:, :W],
                            scalar=float(1.0 / YSCALE), in1=scr1[:, :W],
                            op0=A.mult, op1=A.mult)
                        # w = clip(num / (gap + 1e-9), 0, 1); num = filled offs
                        scr1b = qp.tile([P, QBINS], F32, tag="scr1")
                        nc.vector.tensor_scalar(out=scr1b[:, :W],
                                                in0=filled["dx"][:, fsl],
                                                scalar1=float(1.0 / DXSCALE),
                                                scalar2=1e-9,
                                                op0=A.mult, op1=A.add)
                        scr2 = qp.tile([P, QBINS], F32, tag="scr2")
                        nc.vector.reciprocal(scr2[:, :W], scr1b[:, :W])
                        w_t = qp.tile([P, QBINS], F32, tag="w_t")
                        nc.vector.tensor_tensor(out=w_t[:, :W],
                                                in0=filled["offs"][:, fsl],
                                                in1=scr2[:, :W], op=A.mult)
                        nc.vector.tensor_scalar(out=w_t[:, :W], in0=w_t[:, :W],
                                                scalar1=1.0, scalar2=0.0,
                                                op0=A.min, op1=A.max)
                        for ynm, yc in [("y1", y1c), ("y2", y2c)]:
                            e = qp.tile([P, QBINS], F32, tag="scr2")
                            nc.vector.tensor_tensor(out=e[:, :W],
                                                    in0=filled[ynm + "n"][:, fsl],
                                                    in1=filled[ynm][:, fsl],
                                                    op=A.subtract)
                            nc.vector.tensor_tensor(out=e[:, :W], in0=w_t[:, :W],
                                                    in1=e[:, :W], op=A.mult)
                            nc.vector.tensor_tensor(out=e[:, :W], in0=e[:, :W],
                                                    in1=filled[ynm][:, fsl],
                                                    op=A.add)
                            nc.vector.tensor_tensor(out=yc[:, qb0:qb0 + W],
                                                    in0=e[:, :W], in1=ma[:, :W],
                                                    op=A.mult)
                            spt = sp.tile([P, 1], F32, tag="spt")
                            e2 = qp.tile([P, QBINS], F32, tag="scr2")
                            nc.scalar.activation(e2[:, :W], yc[:, qb0:qb0 + W],
                                                 AF.Square, accum_out=spt[:, 0:1])
                            nc.vector.tensor_tensor(out=sqacc[ynm][:],
                                                    in0=sqacc[ynm][:],
                                                    in1=spt[:], op=A.add)

            # ---- sq = mean(y^2) ------------------------------------------
            sqa = {}
            for ynm in ("y1", "y2"):
                s = pers.tile([P, 1], F32, tag=f"sqa_{ynm}")
                nc.vector.tensor_scalar(out=s[:], in0=sqacc[ynm][:],
                                        scalar1=float(1.0 / M), scalar2=None,
                                        op0=A.mult)
                sqa[ynm] = s

            if debug:
                nc.sync.dma_start(out=dbg["d_y1c"][:], in_=y1c[:, 0:M])
                nc.sync.dma_start(out=dbg["d_y2c"][:], in_=y2c[:, 0:M])
                nc.sync.dma_start(out=dbg["d_sq1"][:], in_=sqa["y1"][:])

            with (
                tc.tile_pool(name="ep", bufs=1) as ep,
                tc.tile_pool(name="rhsp", bufs=3) as rhsp,
            ):
                # ---- transposes to [m, rows] bf16 ------------------------
                ident = ep.tile([P, P], BF16, tag="ident")
                nc.sync.dma_start(out=ident[:], in_=id_in[:])
                y1T = ep.tile([P, MT], BF16, tag="y1T")
                y2T = ep.tile([P, MT], BF16, tag="y2T")
                for kt in range(KT):
                    for src, dstt in [(y1c, y1T), (y2c, y2T)]:
                        ps = pp.tile([P, P], BF16, tag="tps", space="PSUM")
                        nc.tensor.transpose(out=ps[:],
                                            in_=src[:, kt * P:(kt + 1) * P],
                                            identity=ident[:])
                        nc.vector.tensor_copy(out=dstt[:, kt * P:(kt + 1) * P],
                                              in_=ps[:])

                # ---- AllGather of y2T + sq2 hi/res (bf16) ----------------
                # sq2 rides along transposed to the free dim (rows 0/1, 128
                # cols) so the post-gather broadcast DMA reads contiguous
                # 256B chunks instead of 2B strided elements.
                sq2pair = ep.tile([P, 2], BF16, tag="sq2pair")
                nc.vector.tensor_copy(out=sq2pair[:, 0:1], in_=sqa["y2"][:])
                sq2hf = ep.tile([P, 1], F32, tag="sq2hf")
                nc.vector.tensor_copy(out=sq2hf[:], in_=sq2pair[:, 0:1])
                nc.vector.tensor_tensor(out=sq2pair[:, 1:2], in0=sqa["y2"][:],
                                        in1=sq2hf[:], op=A.subtract)
                sqps = pp.tile([2, P], BF16, tag="sqps", space="PSUM")
                nc.tensor.transpose(out=sqps[:], in_=sq2pair[:],
                                    identity=ident[:])
                sq2T = ep.tile([2, P], BF16, tag="sq2T")
                nc.vector.tensor_copy(out=sq2T[:], in_=sqps[:])
                AGW = MT + P  # 3200 per partition-row
                agin = dp.tile([P, AGW], BF16)
                agout = dp.tile([NCORES * P, AGW], BF16, addr_space="Shared")
                nc.sync.dma_start(out=agin[:, 0:MT], in_=y2T[:])
                nc.sync.dma_start(out=agin[0:2, MT:MT + P], in_=sq2T[:])
                nc.gpsimd.collective_compute(
                    "AllGather", A.bypass,
                    replica_groups=[list(range(NCORES))],
                    ins=[agin[:].opt()], outs=[agout[:].opt()])

                # ---- matmul: cross[i,j] = sum_m y1c[i,m]*y2c_all[j,m] ----
                cross = mmpp.tile([P, B], F32, space="PSUM")
                agv = agout[:].rearrange("(r p) f -> r p f", r=NCORES)
                for kt in range(KT):
                    rhs = rhsp.tile([P, B], BF16, tag="rhs")
                    nc.sync.dma_start(
                        out=rhs[:].rearrange("p (r f) -> p r f", r=NCORES),
                        in_=agv[:, :, kt * P:(kt + 1) * P]
                            .rearrange("r p f -> p r f"))
                    for jh in range(2):
                        nc.tensor.matmul(
                            cross[:, jh * 512:(jh + 1) * 512],
                            y1T[:, kt * P:(kt + 1) * P],
                            rhs[:, jh * 512:(jh + 1) * 512],
                            start=(kt == 0), stop=(kt == KT - 1))

                # ---- epilogue --------------------------------------------
                sq2hi_b = ep.tile([P, B], BF16, tag="sq2hi_b")
                nc.sync.dma_start(
                    out=sq2hi_b[:].rearrange("p (r f) -> p r f", r=NCORES),
                    in_=agv[:, 0:1, MT:MT + P].rearrange("r p f -> p r f")
                        .to_broadcast([P, NCORES, P]))
                sq2res_b = ep.tile([P, B], BF16, tag="sq2res_b")
                nc.sync.dma_start(
                    out=sq2res_b[:].rearrange("p (r f) -> p r f", r=NCORES),
                    in_=agv[:, 1:2, MT:MT + P].rearrange("r p f -> p r f")
                        .to_broadcast([P, NCORES, P]))
                sq2g = ep.tile([P, B], F32, tag="sq2g")
                nc.vector.tensor_tensor(out=sq2g[:], in0=sq2hi_b[:],
                                        in1=sq2res_b[:], op=A.add)
                diff = ep.tile([P, B], F32, tag="diff")
                nc.vector.scalar_tensor_tensor(out=diff[:], in0=cross[:],
                                               scalar=float(-2.0 / M),
                                               in1=sq2g[:], op0=A.mult,
                                               op1=A.add)
                nc.vector.tensor_scalar(out=diff[:], in0=diff[:],
                                        scalar1=sqa["y1"][:, 0:1],
                                        scalar2=0.0, op0=A.add, op1=A.max)
                base = ep.tile([P, 1], F32, tag="base")
                nc.vector.tensor_tensor(out=base[:], in0=sqa["y1"][:],
                                        in1=sqa["y2"][:], op=A.add)
                nc.vector.tensor_scalar(out=base[:], in0=base[:], scalar1=1e-8,
                                        scalar2=None, op0=A.add)
                rbase = ep.tile([P, 1], F32, tag="rbase")
                nc.vector.reciprocal(rbase[:], base[:])
                nc.vector.scalar_tensor_tensor(out=diff[:], in0=diff[:],
                                               scalar=2.0,
                                               in1=rbase[:].to_broadcast([P, B]),
                                               op0=A.mult, op1=A.mult)
                lout = ep.tile([P, B], F32, tag="lout")
                nc.scalar.activation(lout[:], diff[:], AF.Sqrt)
                nc.sync.dma_start(out=o_out[:], in_=lout[:])

    nc.compile()
    return nc


def _host_prep(x):
    xmin = np.float32(x[:, 0].min())
    xmax = np.float32(x[:, -1].max())
    grid = np.linspace(np.float32(0.0), np.float32(1.0), M, dtype=np.float32)
    xc = (xmin + grid * (xmax - xmin)).astype(np.float32)[None, :]
    return xmin, xmax, xc


def kernel(x, y1, y2, debug=False, trace=False):
    import ml_dtypes
    from concourse.bass_utils import run_bass_kernel_spmd

    x = np.ascontiguousarray(x, dtype=np.float32)
    y1 = np.ascontiguousarray(y1, dtype=np.float32)
    y2 = np.ascontiguousarray(y2, dtype=np.float32)
    xmin, xmax, xc = _host_prep(x)
    ident = np.eye(P, dtype=ml_dtypes.bfloat16)

    nc = build_nc(float(xmin), float(xmax), debug=debug)
    in_maps = []
    for r in range(NCORES):
        rows = slice(r * R, (r + 1) * R)
        in_maps.append({"x": x[rows], "y1": y1[rows], "y2": y2[rows],
                        "xc": xc, "ident": ident})
    res = run_bass_kernel_spmd(nc, in_maps, core_ids=list(range(NCORES)),
                               trace=trace)
    out = np.concatenate([res.results[r]["out"] for r in range(NCORES)], axis=0)
    if debug or trace:
        return out, res
    return out

